# revision 1
# baseline (speedup 1.0000x reference)
"""Trainium2 Bass kernel for ComplexTVDenoiser (PDHG TV denoising).

Self-contained: kernel(**inputs) takes full inputs {"y": (8,512,512) f32,
"ths": () f32}, shards the batch across 8 NeuronCores (1 image/core),
runs 50 PDHG iterations fully SBUF-resident, returns (8,512,512) f32.

Math (per image, reformulated to update x2 directly):
  e = 1-rho+rho/(1+tau); b2 = -rho*tau/(1+tau); yc = rho*tau/(1+tau)
  q1 = shift_down(u2h) - u2h          (h-adjoint; TensorE matmuls)
  q2 = shift_right(u2w) - u2w         (w-adjoint; DVE shifted APs, guard pads)
  x2' = e*x2 + b2*(q1+q2) + yc*y
  z   = (1-2/rho)*x2 + (2/rho)*x2'
  vh  = u2h + sigma*(shift_up(z) - z)     (TensorE matmuls)
  vw  = u2w + sigma*(shift_left(z) - z)   (DVE)
  f   = ths / sqrt(max(vh^2+vw^2, ths^2)) = exp(-0.5*ln(max(.,ths^2)) + ln ths)
  u2h' = (1-rho)*u2h + rho*f*vh;  u2w' likewise

Layout per core: image rows h = 128*gb + p (p=partition, gb=global block
0..3), organized as 2 chunks x 2 blocks. Per-chunk tensors [128, 2, 512];
tensors read with w-shifts (u2w, z) are padded [128, 2, 516] with data at
[:, :, 1:513] and zero guard columns (makes the w-shift ops exact with no
fixup instructions). h-direction shifts cross partitions, which engines
cannot do (non-zero partition bases are limited to <=32 rows), so they run
as TensorE matmuls with bidiagonal stationary matrices; the block-boundary
rows are accumulated into PSUM with selector matrices.

Optional TVD_BF16=1: u/z subsystem in bf16 (DVE tensor_tensor at 2x,
TensorE at full rate instead of fp32's quarter rate); x2 state and the
x-update chain stay fp32. Accuracy ~2e-3 rel vs 4e-6 for full fp32.
"""
import os
import sys
sys.path.insert(0, "/opt/trn_rl_repo")
sys.path.insert(0, "/opt/trn_rl_repo/concourse")

import numpy as np
import concourse.bass as bass
import concourse.bacc as bacc
import concourse.mybir as mybir
from concourse.tile import TileContext
from concourse.bass_utils import run_bass_kernel_spmd

F32 = mybir.dt.float32
BF16 = mybir.dt.bfloat16
AF = mybir.ActivationFunctionType
OP = mybir.AluOpType

TAU = 0.01
SIGMA = 1.0 / TAU / 8.0
RHO = 1.99
N_IT = 50

E_ = 1.0 - RHO + RHO / (1.0 + TAU)
B2 = -RHO * TAU / (1.0 + TAU)
YC = RHO * TAU / (1.0 + TAU)
ZA = 1.0 - 2.0 / RHO
ZB = 2.0 / RHO

P = 128
W = 512
NCH = 2   # chunks
BPC = 2   # blocks per chunk
WS = 516  # padded block stride

USE_BF16 = os.environ.get("TVD_BF16", "0") == "1"


def _consts(np_dtype):
    madj = (B2 * (np.eye(P, k=1) - np.eye(P)))
    mfwd = (SIGMA * (np.eye(P, k=-1) - np.eye(P)))
    mfwd_last = mfwd.copy()
    mfwd_last[:, P - 1] = 0.0
    # boundary selectors: eadj[127,0]=b2 (row0 of blk gb += b2*row127 of gb-1)
    #                     efwd[0,127]=sigma (row127 of gb += sigma*row0 of gb+1)
    eadj = np.zeros((P, P))
    eadj[P - 1, 0] = B2
    efwd = np.zeros((P, P))
    efwd[0, P - 1] = SIGMA
    ident = np.eye(P)
    cst = np.concatenate([madj, mfwd, mfwd_last, eadj, efwd, ident], axis=1)
    return np.ascontiguousarray(cst.astype(np_dtype))


def build(n_it=N_IT, use_bf16=USE_BF16):
    DT = BF16 if use_bf16 else F32
    nc = bacc.Bacc(None, target_bir_lowering=False)
    y_d = nc.dram_tensor("y", [512, 512], F32, kind="ExternalInput")
    ths_d = nc.dram_tensor("ths", [1, 1], F32, kind="ExternalInput")
    cst_d = nc.dram_tensor("consts", [P, 6 * P], DT, kind="ExternalInput")
    one_d = nc.dram_tensor("onesrow", [1, P], F32, kind="ExternalInput")
    out_d = nc.dram_tensor("out", [512, 512], F32, kind="ExternalOutput")

    with TileContext(nc) as tc:
        with (
            tc.tile_pool(name="st", bufs=1) as st,
            tc.tile_pool(name="ps", bufs=4, space="PSUM") as ps,
        ):
            def T(name, dt, padded=False):
                shape = [P, BPC, WS] if padded else [P, BPC, W]
                return [st.tile(shape, dt, name=f"{name}{c}", tag=f"{name}{c}")
                        for c in range(NCH)]

            x2 = T("x2a", F32)
            x2o = T("x2b", F32)
            u2h = T("u2h", DT)
            u2w = T("u2w", DT, padded=True)
            z = T("z", DT, padded=True)
            yt = T("yt", DT)
            q2 = T("q2", DT)
            s1 = T("s1", DT)
            s2 = T("s2", F32)
            t1 = T("t1", F32)
            d_ = T("dg", DT)
            vw = T("vw", DT)
            vh = None if use_bf16 else T("vh", F32)
            hh = T("hh", DT)
            ww = T("ww", DT)
            n2 = T("n2", DT)
            m_ = T("mm", DT)
            tl = T("tl", F32)
            f_ = T("ff", DT)
            cst = st.tile([P, 6 * P], DT, name="cst", tag="cst")
            ones = st.tile([1, P], F32, name="ones", tag="ones")
            thss = st.tile([1, 1], F32, name="thss", tag="thss")
            thsb = st.tile([P, 1], F32, name="thsb", tag="thsb")
            ths2 = st.tile([P, 1], F32, name="ths2", tag="ths2")
            lnt = st.tile([P, 1], F32, name="lnt", tag="lnt")

            madj = cst[:, 0:P]
            mfwd = cst[:, P:2 * P]
            mfwdl = cst[:, 2 * P:3 * P]
            eadj = cst[:, 3 * P:4 * P]
            efwd = cst[:, 4 * P:5 * P]
            ident = cst[:, 5 * P:6 * P]

            # ---- init ----
            nc.sync.dma_start(out=cst, in_=cst_d[:, :])
            nc.sync.dma_start(out=ones, in_=one_d[:, :])
            nc.sync.dma_start(out=thss, in_=ths_d[:, :])
            for c in range(NCH):
                for b in range(BPC):
                    gb = BPC * c + b
                    nc.sync.dma_start(out=x2[c][:, b, :],
                                      in_=y_d[P * gb:P * (gb + 1), :])
            for c in range(NCH):
                nc.vector.memset(u2h[c], 0.0)
                nc.vector.memset(u2w[c], 0.0)
                nc.vector.memset(z[c], 0.0)
                nc.vector.memset(vw[c], 0.0)  # col 511 is never written later
                nc.scalar.mul(yt[c], x2[c], YC)

            # ths broadcast via K=1 matmul, then ths^2 and ln(ths)
            pb = ps.tile([P, 1], F32, name="pb", tag="pspool")
            nc.tensor.matmul(pb, lhsT=ones, rhs=thss, start=True, stop=True)
            nc.vector.tensor_copy(out=thsb, in_=pb)
            nc.vector.tensor_mul(out=ths2, in0=thsb, in1=thsb)
            nc.scalar.activation(out=lnt, in_=thsb, func=AF.Ln)

            def cb(gb):
                return gb // BPC, gb % BPC

            # ---- iterations ----
            for it in range(n_it):
                psA = [ps.tile([P, BPC, W], F32, name=f"psA{c}_{it}", tag="pspool")
                       for c in range(NCH)]
                # h-adjoint: psA = b2*(shift_down - I) @ u2h (+ boundary rows)
                for c in range(NCH):
                    for b in range(BPC):
                        gb = BPC * c + b
                        nc.tensor.matmul(psA[c][:, b, :], lhsT=madj,
                                         rhs=u2h[c][:, b, :],
                                         start=True, stop=(gb == 0))
                        if gb > 0:
                            sc_, sb_ = cb(gb - 1)
                            nc.tensor.matmul(psA[c][:, b, :], lhsT=eadj,
                                             rhs=u2h[sc_][:, sb_, :],
                                             start=False, stop=True)
                for c in range(NCH):
                    # w-adjoint: q2 = shift_right(u2w) - u2w (guards make it exact)
                    nc.vector.tensor_sub(out=q2[c], in0=u2w[c][:, :, 0:W],
                                         in1=u2w[c][:, :, 1:W + 1])
                for c in range(NCH):
                    nc.vector.scalar_tensor_tensor(
                        out=s1[c], in0=q2[c], scalar=B2, in1=yt[c],
                        op0=OP.mult, op1=OP.add)
                for c in range(NCH):
                    nc.vector.scalar_tensor_tensor(
                        out=s2[c], in0=x2[c], scalar=E_, in1=psA[c],
                        op0=OP.mult, op1=OP.add)
                for c in range(NCH):
                    nc.gpsimd.tensor_add(out=x2o[c], in0=s1[c], in1=s2[c])
                for c in range(NCH):
                    nc.scalar.mul(t1[c], x2[c], ZA)
                for c in range(NCH):
                    nc.vector.scalar_tensor_tensor(
                        out=z[c][:, :, 1:W + 1], in0=x2o[c], scalar=ZB,
                        in1=t1[c], op0=OP.mult, op1=OP.add)

                psV = [ps.tile([P, BPC, W], F32, name=f"psV{c}_{it}", tag="pspool")
                       for c in range(NCH)]
                # h-forward: psV = sigma*(shift_up - I) @ z (+ boundary rows;
                # in bf16 mode also folds + I @ u2h so psV becomes vh)
                last = NCH * BPC - 1
                for c in range(NCH):
                    for b in range(BPC):
                        gb = BPC * c + b
                        group = [((mfwdl if gb == last else mfwd),
                                  z[c][:, b, 1:W + 1])]
                        if use_bf16:
                            group.append((ident, u2h[c][:, b, :]))
                        if gb < last:
                            sc_, sb_ = cb(gb + 1)
                            group.append((efwd, z[sc_][:, sb_, 1:W + 1]))
                        for gi, (lhsT, rhs) in enumerate(group):
                            nc.tensor.matmul(psV[c][:, b, :], lhsT=lhsT,
                                             rhs=rhs, start=(gi == 0),
                                             stop=(gi == len(group) - 1))
                for c in range(NCH):
                    # w-gradient (w=0..510): d = shift_left(z) - z
                    nc.vector.tensor_sub(out=d_[c][:, :, 0:W - 1],
                                         in0=z[c][:, :, 2:W + 1],
                                         in1=z[c][:, :, 1:W])
                for c in range(NCH):
                    nc.vector.scalar_tensor_tensor(
                        out=vw[c][:, :, 0:W - 1], in0=d_[c][:, :, 0:W - 1],
                        scalar=SIGMA, in1=u2w[c][:, :, 1:W],
                        op0=OP.mult, op1=OP.add)
                if use_bf16:
                    vhsrc = psV
                else:
                    for c in range(NCH):
                        nc.vector.tensor_add(out=vh[c], in0=psV[c], in1=u2h[c])
                    vhsrc = vh
                for c in range(NCH):
                    nc.scalar.activation(out=hh[c], in_=vhsrc[c], func=AF.Square)
                for c in range(NCH):
                    nc.scalar.activation(out=ww[c], in_=vw[c], func=AF.Square)
                for c in range(NCH):
                    nc.gpsimd.tensor_add(out=n2[c], in0=hh[c], in1=ww[c])
                for c in range(NCH):
                    nc.vector.tensor_scalar(out=m_[c], in0=n2[c],
                                            scalar1=ths2[:, 0:1], scalar2=None,
                                            op0=OP.max)
                for c in range(NCH):
                    nc.scalar.activation(out=tl[c], in_=m_[c], func=AF.Ln)
                for c in range(NCH):
                    nc.scalar.activation(out=f_[c], in_=tl[c], func=AF.Exp,
                                         bias=lnt[:, 0:1], scale=-0.5)
                # ph/pw reuse the hh/ww buffers (dead after n2)
                for c in range(NCH):
                    nc.vector.scalar_tensor_tensor(
                        out=hh[c], in0=vhsrc[c], scalar=RHO, in1=f_[c],
                        op0=OP.mult, op1=OP.mult)
                for c in range(NCH):
                    nc.vector.scalar_tensor_tensor(
                        out=u2h[c], in0=u2h[c], scalar=1.0 - RHO,
                        in1=hh[c], op0=OP.mult, op1=OP.add)
                for c in range(NCH):
                    nc.vector.scalar_tensor_tensor(
                        out=ww[c], in0=vw[c], scalar=RHO, in1=f_[c],
                        op0=OP.mult, op1=OP.mult)
                for c in range(NCH):
                    nc.vector.scalar_tensor_tensor(
                        out=u2w[c][:, :, 1:W + 1], in0=u2w[c][:, :, 1:W + 1],
                        scalar=1.0 - RHO, in1=ww[c],
                        op0=OP.mult, op1=OP.add)
                x2, x2o = x2o, x2

            # ---- writeback ----
            for c in range(NCH):
                for b in range(BPC):
                    gb = BPC * c + b
                    nc.sync.dma_start(out=out_d[P * gb:P * (gb + 1), :],
                                      in_=x2[c][:, b, :])
    nc.compile()
    return nc


_CACHED = {}


def kernel(y: np.ndarray, ths: np.ndarray, n_it=N_IT) -> np.ndarray:
    y = np.ascontiguousarray(np.asarray(y, dtype=np.float32))
    B = y.shape[0]
    assert y.shape[1:] == (512, 512), y.shape
    key = ("nc", n_it, USE_BF16)
    if key not in _CACHED:
        import time as _t
        _tb = _t.time()
        _CACHED[key] = build(n_it)
        print(f"[kernel] build({n_it}) took {_t.time()-_tb:.1f}s", flush=True)
    nc = _CACHED[key]
    if USE_BF16:
        import ml_dtypes
        cst = _consts(ml_dtypes.bfloat16)
    else:
        cst = _consts(np.float32)
    onesrow = np.ones((1, P), dtype=np.float32)
    thsv = np.asarray(ths, dtype=np.float32).reshape(1, 1)
    in_maps = [{"y": y[i], "ths": thsv, "consts": cst, "onesrow": onesrow}
               for i in range(B)]
    trace = bool(os.environ.get("TVD_TRACE"))
    import time as _t
    _tr = _t.time()
    res = run_bass_kernel_spmd(nc, in_maps, core_ids=list(range(B)),
                               trace=trace)
    print(f"[kernel] run took {_t.time()-_tr:.1f}s", flush=True)
    _CACHED["last_res"] = res
    out = np.stack([res.results[i]["out"] for i in range(B)])
    return out.astype(np.float32)


if __name__ == "__main__":
    rng = np.random.default_rng(0)
    y = rng.standard_normal((8, 512, 512), dtype=np.float32)
    out = kernel(y, np.float32(0.1))
    print("ran:", out.shape, out.dtype, float(np.abs(out).max()))



# revision 4
# speedup vs baseline: 99.6365x; 99.6365x over previous
"""Trainium2 Bass kernel for ComplexTVDenoiser (PDHG TV denoising).

Self-contained: kernel(**inputs) takes full inputs {"y": (8,512,512) f32,
"ths": () f32}, shards the batch across 8 NeuronCores (1 image/core),
runs 50 PDHG iterations fully SBUF-resident, returns (8,512,512) f32.

Math (per image, reformulated to update x2 directly):
  e = 1-rho+rho/(1+tau); b2 = -rho*tau/(1+tau); yc = rho*tau/(1+tau)
  q1 = shift_down(u2h) - u2h          (h-adjoint; TensorE matmuls)
  q2 = shift_right(u2w) - u2w         (w-adjoint; DVE shifted APs, guard pads)
  x2' = e*x2 + b2*(q1+q2) + yc*y
  z   = (1-2/rho)*x2 + (2/rho)*x2'
  vh  = u2h + sigma*(shift_up(z) - z)     (TensorE matmuls)
  vw  = u2w + sigma*(shift_left(z) - z)   (DVE)
  f   = ths / sqrt(max(vh^2+vw^2, ths^2)) = exp(-0.5*ln(max(.,ths^2)) + ln ths)
  u2h' = (1-rho)*u2h + rho*f*vh;  u2w' likewise

Layout per core: image rows h = 128*gb + p (p=partition, gb=global block
0..3), organized as 2 chunks x 2 blocks. Per-chunk tensors [128, 2, 512];
tensors read with w-shifts (u2w, z) are padded [128, 2, 516] with data at
[:, :, 1:513] and zero guard columns (makes the w-shift ops exact with no
fixup instructions). h-direction shifts cross partitions, which engines
cannot do (non-zero partition bases are limited to <=32 rows), so they run
as TensorE matmuls with bidiagonal stationary matrices; the block-boundary
rows are accumulated into PSUM with selector matrices.

Optional TVD_BF16=1: u/z subsystem in bf16 (DVE tensor_tensor at 2x,
TensorE at full rate instead of fp32's quarter rate); x2 state and the
x-update chain stay fp32. Accuracy ~2e-3 rel vs 4e-6 for full fp32.
"""
import os
import sys
sys.path.insert(0, "/opt/trn_rl_repo")
sys.path.insert(0, "/opt/trn_rl_repo/concourse")

import numpy as np
import concourse.bass as bass
import concourse.bacc as bacc
import concourse.mybir as mybir
from concourse.tile import TileContext
from concourse.bass_utils import run_bass_kernel_spmd

F32 = mybir.dt.float32
BF16 = mybir.dt.bfloat16
AF = mybir.ActivationFunctionType
OP = mybir.AluOpType

TAU = 0.01
SIGMA = 1.0 / TAU / 8.0
RHO = 1.99
N_IT = 50

E_ = 1.0 - RHO + RHO / (1.0 + TAU)
B2 = -RHO * TAU / (1.0 + TAU)
YC = RHO * TAU / (1.0 + TAU)
ZA = 1.0 - 2.0 / RHO
ZB = 2.0 / RHO

P = 128
W = 512
NCH = 2   # chunks
BPC = 2   # blocks per chunk
WS = 516  # padded block stride

USE_BF16 = os.environ.get("TVD_BF16", "0") == "1"


def _consts(np_dtype):
    madj = (B2 * (np.eye(P, k=1) - np.eye(P)))
    mfwd = (SIGMA * (np.eye(P, k=-1) - np.eye(P)))
    mfwd_last = mfwd.copy()
    mfwd_last[:, P - 1] = 0.0
    # boundary selectors: eadj[127,0]=b2 (row0 of blk gb += b2*row127 of gb-1)
    #                     efwd[0,127]=sigma (row127 of gb += sigma*row0 of gb+1)
    eadj = np.zeros((P, P))
    eadj[P - 1, 0] = B2
    efwd = np.zeros((P, P))
    efwd[0, P - 1] = SIGMA
    ident = np.eye(P)
    cst = np.concatenate([madj, mfwd, mfwd_last, eadj, efwd, ident], axis=1)
    return np.ascontiguousarray(cst.astype(np_dtype))


def build(n_it=N_IT, use_bf16=USE_BF16):
    DT = BF16 if use_bf16 else F32
    nc = bacc.Bacc(None, target_bir_lowering=False)
    y_d = nc.dram_tensor("y", [512, 512], F32, kind="ExternalInput")
    ths_d = nc.dram_tensor("ths", [1, 1], F32, kind="ExternalInput")
    cst_d = nc.dram_tensor("consts", [P, 6 * P], DT, kind="ExternalInput")
    one_d = nc.dram_tensor("onesrow", [1, P], F32, kind="ExternalInput")
    out_d = nc.dram_tensor("out", [512, 512], F32, kind="ExternalOutput")

    with TileContext(nc) as tc:
        with (
            tc.tile_pool(name="st", bufs=1) as st,
            tc.tile_pool(name="ps", bufs=4, space="PSUM") as ps,
        ):
            def T(name, dt, padded=False):
                shape = [P, BPC, WS] if padded else [P, BPC, W]
                return [st.tile(shape, dt, name=f"{name}{c}", tag=f"{name}{c}")
                        for c in range(NCH)]

            x2 = T("x2a", F32)
            x2o = T("x2b", F32)
            u2h = T("u2h", DT)
            u2w = T("u2w", DT, padded=True)
            z = T("z", DT, padded=True)
            yt = T("yt", DT)
            q2 = T("q2", DT)
            s1 = T("s1", DT)
            s2 = T("s2", F32)
            t1 = T("t1", F32)
            d_ = T("dg", DT)
            vw = T("vw", DT)
            vh = None if use_bf16 else T("vh", F32)
            hh = T("hh", DT)
            ww = T("ww", DT)
            n2 = T("n2", DT)
            m_ = T("mm", DT)
            tl = T("tl", F32)
            f_ = T("ff", DT)
            cst = st.tile([P, 6 * P], DT, name="cst", tag="cst")
            ones = st.tile([1, P], F32, name="ones", tag="ones")
            thss = st.tile([1, 1], F32, name="thss", tag="thss")
            thsb = st.tile([P, 1], F32, name="thsb", tag="thsb")
            ths2 = st.tile([P, 1], F32, name="ths2", tag="ths2")
            lnt = st.tile([P, 1], F32, name="lnt", tag="lnt")

            madj = cst[:, 0:P]
            mfwd = cst[:, P:2 * P]
            mfwdl = cst[:, 2 * P:3 * P]
            eadj = cst[:, 3 * P:4 * P]
            efwd = cst[:, 4 * P:5 * P]
            ident = cst[:, 5 * P:6 * P]

            # ---- init ----
            nc.sync.dma_start(out=cst, in_=cst_d[:, :])
            nc.sync.dma_start(out=ones, in_=one_d[:, :])
            nc.sync.dma_start(out=thss, in_=ths_d[:, :])
            for c in range(NCH):
                for b in range(BPC):
                    gb = BPC * c + b
                    nc.sync.dma_start(out=x2[c][:, b, :],
                                      in_=y_d[P * gb:P * (gb + 1), :])
            for c in range(NCH):
                nc.vector.memset(u2h[c], 0.0)
                nc.vector.memset(u2w[c], 0.0)
                nc.vector.memset(z[c], 0.0)
                nc.vector.memset(vw[c], 0.0)  # col 511 is never written later
                nc.scalar.mul(yt[c], x2[c], YC)

            # ths broadcast via K=1 matmul, then ths^2 and ln(ths)
            pb = ps.tile([P, 1], F32, name="pb", tag="pspool")
            nc.tensor.matmul(pb, lhsT=ones, rhs=thss, start=True, stop=True)
            nc.vector.tensor_copy(out=thsb, in_=pb)
            nc.vector.tensor_mul(out=ths2, in0=thsb, in1=thsb)
            nc.scalar.activation(out=lnt, in_=thsb, func=AF.Ln)

            def cb(gb):
                return gb // BPC, gb % BPC

            # ---- one PDHG iteration: reads x2, writes x2o ----
            def half_iter(x2, x2o, it):
                psA = [ps.tile([P, BPC, W], F32, name=f"psA{c}_{it}", tag="pspool")
                       for c in range(NCH)]
                # h-adjoint: psA = b2*(shift_down - I) @ u2h (+ boundary rows)
                for c in range(NCH):
                    for b in range(BPC):
                        gb = BPC * c + b
                        nc.tensor.matmul(psA[c][:, b, :], lhsT=madj,
                                         rhs=u2h[c][:, b, :],
                                         start=True, stop=(gb == 0))
                        if gb > 0:
                            sc_, sb_ = cb(gb - 1)
                            nc.tensor.matmul(psA[c][:, b, :], lhsT=eadj,
                                             rhs=u2h[sc_][:, sb_, :],
                                             start=False, stop=True)
                for c in range(NCH):
                    # w-adjoint: q2 = shift_right(u2w) - u2w (guards make it exact)
                    nc.vector.tensor_sub(out=q2[c], in0=u2w[c][:, :, 0:W],
                                         in1=u2w[c][:, :, 1:W + 1])
                for c in range(NCH):
                    nc.vector.scalar_tensor_tensor(
                        out=s1[c], in0=q2[c], scalar=B2, in1=yt[c],
                        op0=OP.mult, op1=OP.add)
                for c in range(NCH):
                    nc.vector.scalar_tensor_tensor(
                        out=s2[c], in0=x2[c], scalar=E_, in1=psA[c],
                        op0=OP.mult, op1=OP.add)
                for c in range(NCH):
                    nc.gpsimd.tensor_add(out=x2o[c], in0=s1[c], in1=s2[c])
                for c in range(NCH):
                    nc.scalar.mul(t1[c], x2[c], ZA)
                for c in range(NCH):
                    nc.vector.scalar_tensor_tensor(
                        out=z[c][:, :, 1:W + 1], in0=x2o[c], scalar=ZB,
                        in1=t1[c], op0=OP.mult, op1=OP.add)

                psV = [ps.tile([P, BPC, W], F32, name=f"psV{c}_{it}", tag="pspool")
                       for c in range(NCH)]
                # h-forward: psV = sigma*(shift_up - I) @ z (+ boundary rows;
                # in bf16 mode also folds + I @ u2h so psV becomes vh)
                last = NCH * BPC - 1
                for c in range(NCH):
                    for b in range(BPC):
                        gb = BPC * c + b
                        group = [((mfwdl if gb == last else mfwd),
                                  z[c][:, b, 1:W + 1])]
                        if use_bf16:
                            group.append((ident, u2h[c][:, b, :]))
                        if gb < last:
                            sc_, sb_ = cb(gb + 1)
                            group.append((efwd, z[sc_][:, sb_, 1:W + 1]))
                        for gi, (lhsT, rhs) in enumerate(group):
                            nc.tensor.matmul(psV[c][:, b, :], lhsT=lhsT,
                                             rhs=rhs, start=(gi == 0),
                                             stop=(gi == len(group) - 1))
                for c in range(NCH):
                    # w-gradient (w=0..510): d = shift_left(z) - z
                    nc.vector.tensor_sub(out=d_[c][:, :, 0:W - 1],
                                         in0=z[c][:, :, 2:W + 1],
                                         in1=z[c][:, :, 1:W])
                for c in range(NCH):
                    nc.vector.scalar_tensor_tensor(
                        out=vw[c][:, :, 0:W - 1], in0=d_[c][:, :, 0:W - 1],
                        scalar=SIGMA, in1=u2w[c][:, :, 1:W],
                        op0=OP.mult, op1=OP.add)
                if use_bf16:
                    vhsrc = psV
                else:
                    for c in range(NCH):
                        nc.vector.tensor_add(out=vh[c], in0=psV[c], in1=u2h[c])
                    vhsrc = vh
                for c in range(NCH):
                    nc.scalar.activation(out=hh[c], in_=vhsrc[c], func=AF.Square)
                for c in range(NCH):
                    nc.scalar.activation(out=ww[c], in_=vw[c], func=AF.Square)
                for c in range(NCH):
                    nc.gpsimd.tensor_add(out=n2[c], in0=hh[c], in1=ww[c])
                for c in range(NCH):
                    nc.vector.tensor_scalar(out=m_[c], in0=n2[c],
                                            scalar1=ths2[:, 0:1], scalar2=None,
                                            op0=OP.max)
                for c in range(NCH):
                    nc.scalar.activation(out=tl[c], in_=m_[c], func=AF.Ln)
                for c in range(NCH):
                    nc.scalar.activation(out=f_[c], in_=tl[c], func=AF.Exp,
                                         bias=lnt[:, 0:1], scale=-0.5)
                # ph/pw reuse the hh/ww buffers (dead after n2)
                for c in range(NCH):
                    nc.vector.scalar_tensor_tensor(
                        out=hh[c], in0=vhsrc[c], scalar=RHO, in1=f_[c],
                        op0=OP.mult, op1=OP.mult)
                for c in range(NCH):
                    nc.vector.scalar_tensor_tensor(
                        out=u2h[c], in0=u2h[c], scalar=1.0 - RHO,
                        in1=hh[c], op0=OP.mult, op1=OP.add)
                for c in range(NCH):
                    nc.vector.scalar_tensor_tensor(
                        out=ww[c], in0=vw[c], scalar=RHO, in1=f_[c],
                        op0=OP.mult, op1=OP.mult)
                for c in range(NCH):
                    nc.vector.scalar_tensor_tensor(
                        out=u2w[c][:, :, 1:W + 1], in0=u2w[c][:, :, 1:W + 1],
                        scalar=1.0 - RHO, in1=ww[c],
                        op0=OP.mult, op1=OP.add)

            # ---- iterations: hardware loop, 2 PDHG steps per body ----
            assert n_it % 2 == 0, n_it
            if n_it > 0:
                with tc.For_i(0, n_it // 2, 1):
                    half_iter(x2, x2o, 0)
                    half_iter(x2o, x2, 1)

            # ---- writeback ----
            for c in range(NCH):
                for b in range(BPC):
                    gb = BPC * c + b
                    nc.sync.dma_start(out=out_d[P * gb:P * (gb + 1), :],
                                      in_=x2[c][:, b, :])
    nc.compile()
    return nc


_CACHED = {}


def _make_runner(nc, n_cores):
    """Build a reusable jitted executor for nc (mirrors
    bass2jax.run_bass_via_pjrt, but the jax.jit object is created once so
    repeat calls skip retrace/recompile entirely)."""
    import jax
    from jax.experimental.shard_map import shard_map
    from jax.sharding import Mesh, PartitionSpec
    from concourse import bass2jax

    bass2jax.install_neuronx_cc_hook()
    assert nc.dbg_addr is None
    partition_name = (nc.partition_id_tensor.name
                      if nc.partition_id_tensor else None)
    in_names, out_names, out_avals, zero_specs = [], [], [], []
    for alloc in nc.m.functions[0].allocations:
        if not isinstance(alloc, mybir.MemoryLocationSet):
            continue
        name = alloc.memorylocations[0].name
        if alloc.kind == "ExternalInput":
            if name != partition_name:
                in_names.append(name)
        elif alloc.kind == "ExternalOutput":
            shape = tuple(alloc.tensor_shape)
            dtype = mybir.dt.np(alloc.dtype)
            out_names.append(name)
            out_avals.append(jax.core.ShapedArray(shape, dtype))
            zero_specs.append((shape, dtype))
    n_params = len(in_names)
    n_outs = len(out_avals)
    in_names = in_names + out_names
    if partition_name is not None:
        in_names.append(partition_name)
    donate = tuple(range(n_params, n_params + n_outs))

    def _body(*args):
        operands = list(args)
        if partition_name is not None:
            operands.append(bass2jax.partition_id_tensor())
        outs = bass2jax._bass_exec_p.bind(
            *operands,
            out_avals=tuple(out_avals),
            in_names=tuple(in_names),
            out_names=tuple(out_names),
            lowering_input_output_aliases=(),
            sim_require_finite=True,
            sim_require_nnan=True,
            nc=nc,
        )
        return tuple(outs)

    devices = jax.devices()[:n_cores]
    assert len(devices) == n_cores
    mesh = Mesh(np.asarray(devices), ("core",))
    in_specs = (PartitionSpec("core"),) * (n_params + n_outs)
    out_specs = (PartitionSpec("core"),) * n_outs
    sharded = jax.jit(
        shard_map(_body, mesh=mesh, in_specs=in_specs,
                  out_specs=out_specs, check_rep=False),
        donate_argnums=donate, keep_unused=True)

    def run(in_maps):
        per_core = [[np.asarray(m[name]) for name in in_names[:n_params]]
                    for m in in_maps]
        concat_in = [np.concatenate([per_core[c][i] for c in range(n_cores)],
                                    axis=0) for i in range(n_params)]
        concat_zeros = [np.zeros((n_cores * s[0], *s[1:]), d)
                        for (s, d) in zero_specs]
        out_arrs = sharded(*concat_in, *concat_zeros)
        return [{name: np.asarray(out_arrs[i]).reshape(n_cores,
                                                       *out_avals[i].shape)[c]
                 for i, name in enumerate(out_names)}
                for c in range(n_cores)]
    return run


def kernel(y: np.ndarray, ths: np.ndarray, n_it=N_IT) -> np.ndarray:
    y = np.ascontiguousarray(np.asarray(y, dtype=np.float32))
    B = y.shape[0]
    assert y.shape[1:] == (512, 512), y.shape
    key = ("run", n_it, USE_BF16, B)
    if key not in _CACHED:
        import time as _t
        _tb = _t.time()
        nc = build(n_it)
        _CACHED[key] = _make_runner(nc, B)
        print(f"[kernel] build({n_it}) took {_t.time()-_tb:.1f}s", flush=True)
    run = _CACHED[key]
    if USE_BF16:
        import ml_dtypes
        cst = _consts(ml_dtypes.bfloat16)
    else:
        cst = _consts(np.float32)
    onesrow = np.ones((1, P), dtype=np.float32)
    thsv = np.asarray(ths, dtype=np.float32).reshape(1, 1)
    in_maps = [{"y": y[i], "ths": thsv, "consts": cst, "onesrow": onesrow}
               for i in range(B)]
    import time as _t
    _tr = _t.time()
    results = run(in_maps)
    print(f"[kernel] run took {_t.time()-_tr:.1f}s", flush=True)
    out = np.stack([results[i]["out"] for i in range(B)])
    return out.astype(np.float32)


if __name__ == "__main__":
    rng = np.random.default_rng(0)
    y = rng.standard_normal((8, 512, 512), dtype=np.float32)
    out = kernel(y, np.float32(0.1))
    print("ran:", out.shape, out.dtype, float(np.abs(out).max()))



# revision 6
# speedup vs baseline: 115.9880x; 1.1641x over previous
"""Trainium2 Bass kernel for ComplexTVDenoiser (PDHG TV denoising).

Self-contained: kernel(**inputs) takes full inputs {"y": (8,512,512) f32,
"ths": () f32}, shards the batch across 8 NeuronCores (1 image/core),
runs 50 PDHG iterations fully SBUF-resident, returns (8,512,512) f32.

Math (per image, reformulated to update x2 directly):
  e = 1-rho+rho/(1+tau); b2 = -rho*tau/(1+tau); yc = rho*tau/(1+tau)
  q1 = shift_down(u2h) - u2h          (h-adjoint; TensorE matmuls)
  q2 = shift_right(u2w) - u2w         (w-adjoint; DVE shifted APs, guard pads)
  x2' = e*x2 + b2*(q1+q2) + yc*y
  z   = (1-2/rho)*x2 + (2/rho)*x2'
  vh  = u2h + sigma*(shift_up(z) - z)     (TensorE matmuls)
  vw  = u2w + sigma*(shift_left(z) - z)   (DVE)
  f   = ths / sqrt(max(vh^2+vw^2, ths^2)) = exp(-0.5*ln(max(.,ths^2)) + ln ths)
  u2h' = (1-rho)*u2h + rho*f*vh;  u2w' likewise

Layout per core: image rows h = 128*gb + p (p=partition, gb=global block
0..3), organized as 2 chunks x 2 blocks. Per-chunk tensors [128, 2, 512];
tensors read with w-shifts (u2w, z) are padded [128, 2, 516] with data at
[:, :, 1:513] and zero guard columns (makes the w-shift ops exact with no
fixup instructions). h-direction shifts cross partitions, which engines
cannot do (non-zero partition bases are limited to <=32 rows), so they run
as TensorE matmuls with bidiagonal stationary matrices; the block-boundary
rows are accumulated into PSUM with selector matrices.

Optional TVD_BF16=1: u/z subsystem in bf16 (DVE tensor_tensor at 2x,
TensorE at full rate instead of fp32's quarter rate); x2 state and the
x-update chain stay fp32. Accuracy ~2e-3 rel vs 4e-6 for full fp32.
"""
import os
import sys
sys.path.insert(0, "/opt/trn_rl_repo")
sys.path.insert(0, "/opt/trn_rl_repo/concourse")

import numpy as np
import concourse.bass as bass
import concourse.bacc as bacc
import concourse.mybir as mybir
from concourse.tile import TileContext
from concourse.bass_utils import run_bass_kernel_spmd

F32 = mybir.dt.float32
BF16 = mybir.dt.bfloat16
AF = mybir.ActivationFunctionType
OP = mybir.AluOpType

TAU = 0.01
SIGMA = 1.0 / TAU / 8.0
RHO = 1.99
N_IT = 50

E_ = 1.0 - RHO + RHO / (1.0 + TAU)
B2 = -RHO * TAU / (1.0 + TAU)
YC = RHO * TAU / (1.0 + TAU)
ZA = 1.0 - 2.0 / RHO
ZB = 2.0 / RHO

P = 128
W = 512
NCH = 2   # chunks
BPC = 2   # blocks per chunk
WS = 516  # padded block stride

USE_BF16 = os.environ.get("TVD_BF16", "0") == "1"


def _consts(np_dtype):
    madj = (B2 * (np.eye(P, k=1) - np.eye(P)))
    mfwd = (SIGMA * (np.eye(P, k=-1) - np.eye(P)))
    mfwd_last = mfwd.copy()
    mfwd_last[:, P - 1] = 0.0
    # boundary selectors: eadj[127,0]=b2 (row0 of blk gb += b2*row127 of gb-1)
    #                     efwd[0,127]=sigma (row127 of gb += sigma*row0 of gb+1)
    eadj = np.zeros((P, P))
    eadj[P - 1, 0] = B2
    efwd = np.zeros((P, P))
    efwd[0, P - 1] = SIGMA
    ident = np.eye(P)
    cst = np.concatenate([madj, mfwd, mfwd_last, eadj, efwd, ident], axis=1)
    return np.ascontiguousarray(cst.astype(np_dtype))


def build(n_it=N_IT, use_bf16=USE_BF16, unrolled=False):
    DT = BF16 if use_bf16 else F32
    nc = bacc.Bacc(None, target_bir_lowering=False)
    y_d = nc.dram_tensor("y", [512, 512], F32, kind="ExternalInput")
    ths_d = nc.dram_tensor("ths", [1, 1], F32, kind="ExternalInput")
    cst_d = nc.dram_tensor("consts", [P, 6 * P], DT, kind="ExternalInput")
    one_d = nc.dram_tensor("onesrow", [1, P], F32, kind="ExternalInput")
    out_d = nc.dram_tensor("out", [512, 512], F32, kind="ExternalOutput")

    with TileContext(nc) as tc:
        with (
            tc.tile_pool(name="st", bufs=1) as st,
            tc.tile_pool(name="ps", bufs=4, space="PSUM") as ps,
        ):
            def T(name, dt, padded=False):
                shape = [P, BPC, WS] if padded else [P, BPC, W]
                return [st.tile(shape, dt, name=f"{name}{c}", tag=f"{name}{c}")
                        for c in range(NCH)]

            x2 = T("x2a", F32)
            x2o = T("x2b", F32)
            u2h = T("u2h", DT)
            u2w = T("u2w", DT, padded=True)
            z = T("z", DT, padded=True)
            yt = T("yt", DT)
            q2 = T("q2", DT)
            s1 = T("s1", DT)
            s2 = T("s2", F32)
            t1 = T("t1", F32)
            d_ = T("dg", DT)
            vw = T("vw", DT)
            vh = None if use_bf16 else T("vh", F32)
            hh = T("hh", DT)
            ww = T("ww", DT)
            n2 = T("n2", DT)
            m_ = T("mm", DT)
            tl = T("tl", F32)
            f_ = T("ff", DT)
            cst = st.tile([P, 6 * P], DT, name="cst", tag="cst")
            ones = st.tile([1, P], F32, name="ones", tag="ones")
            thss = st.tile([1, 1], F32, name="thss", tag="thss")
            thsb = st.tile([P, 1], F32, name="thsb", tag="thsb")
            ths2 = st.tile([P, 1], F32, name="ths2", tag="ths2")
            lnt = st.tile([P, 1], F32, name="lnt", tag="lnt")

            madj = cst[:, 0:P]
            mfwd = cst[:, P:2 * P]
            mfwdl = cst[:, 2 * P:3 * P]
            eadj = cst[:, 3 * P:4 * P]
            efwd = cst[:, 4 * P:5 * P]
            ident = cst[:, 5 * P:6 * P]

            # ---- init ----
            nc.sync.dma_start(out=cst, in_=cst_d[:, :])
            nc.sync.dma_start(out=ones, in_=one_d[:, :])
            nc.sync.dma_start(out=thss, in_=ths_d[:, :])
            for c in range(NCH):
                for b in range(BPC):
                    gb = BPC * c + b
                    nc.sync.dma_start(out=x2[c][:, b, :],
                                      in_=y_d[P * gb:P * (gb + 1), :])
            for c in range(NCH):
                nc.vector.memset(u2h[c], 0.0)
                nc.vector.memset(u2w[c], 0.0)
                nc.vector.memset(z[c], 0.0)
                nc.vector.memset(vw[c], 0.0)  # col 511 is never written later
                nc.scalar.mul(yt[c], x2[c], YC)

            # ths broadcast via K=1 matmul, then ths^2 and ln(ths)
            pb = ps.tile([P, 1], F32, name="pb", tag="pspool")
            nc.tensor.matmul(pb, lhsT=ones, rhs=thss, start=True, stop=True)
            nc.vector.tensor_copy(out=thsb, in_=pb)
            nc.vector.tensor_mul(out=ths2, in0=thsb, in1=thsb)
            nc.scalar.activation(out=lnt, in_=thsb, func=AF.Ln)

            def cb(gb):
                return gb // BPC, gb % BPC

            # ---- one PDHG iteration: reads x2, writes x2o ----
            def half_iter(x2, x2o, it):
                psA = [ps.tile([P, BPC, W], F32, name=f"psA{c}_{it}", tag="pspool")
                       for c in range(NCH)]
                # h-adjoint: psA = b2*(shift_down - I) @ u2h (+ boundary rows)
                for c in range(NCH):
                    for b in range(BPC):
                        gb = BPC * c + b
                        nc.tensor.matmul(psA[c][:, b, :], lhsT=madj,
                                         rhs=u2h[c][:, b, :],
                                         start=True, stop=(gb == 0))
                        if gb > 0:
                            sc_, sb_ = cb(gb - 1)
                            nc.tensor.matmul(psA[c][:, b, :], lhsT=eadj,
                                             rhs=u2h[sc_][:, sb_, :],
                                             start=False, stop=True)
                for c in range(NCH):
                    # w-adjoint: q2 = shift_right(u2w) - u2w (guards make it exact)
                    nc.vector.tensor_sub(out=q2[c], in0=u2w[c][:, :, 0:W],
                                         in1=u2w[c][:, :, 1:W + 1])
                for c in range(NCH):
                    nc.vector.scalar_tensor_tensor(
                        out=s1[c], in0=q2[c], scalar=B2, in1=yt[c],
                        op0=OP.mult, op1=OP.add)
                for c in range(NCH):
                    nc.vector.scalar_tensor_tensor(
                        out=s2[c], in0=x2[c], scalar=E_, in1=psA[c],
                        op0=OP.mult, op1=OP.add)
                for c in range(NCH):
                    nc.gpsimd.tensor_add(out=x2o[c], in0=s1[c], in1=s2[c])
                for c in range(NCH):
                    nc.scalar.mul(t1[c], x2[c], ZA)
                for c in range(NCH):
                    nc.vector.scalar_tensor_tensor(
                        out=z[c][:, :, 1:W + 1], in0=x2o[c], scalar=ZB,
                        in1=t1[c], op0=OP.mult, op1=OP.add)

                psV = [ps.tile([P, BPC, W], F32, name=f"psV{c}_{it}", tag="pspool")
                       for c in range(NCH)]
                # h-forward: psV = sigma*(shift_up - I) @ z (+ boundary rows;
                # in bf16 mode also folds + I @ u2h so psV becomes vh)
                last = NCH * BPC - 1
                for c in range(NCH):
                    for b in range(BPC):
                        gb = BPC * c + b
                        group = [((mfwdl if gb == last else mfwd),
                                  z[c][:, b, 1:W + 1])]
                        if use_bf16:
                            group.append((ident, u2h[c][:, b, :]))
                        if gb < last:
                            sc_, sb_ = cb(gb + 1)
                            group.append((efwd, z[sc_][:, sb_, 1:W + 1]))
                        for gi, (lhsT, rhs) in enumerate(group):
                            nc.tensor.matmul(psV[c][:, b, :], lhsT=lhsT,
                                             rhs=rhs, start=(gi == 0),
                                             stop=(gi == len(group) - 1))
                for c in range(NCH):
                    # w-gradient (w=0..510): d = shift_left(z) - z
                    nc.vector.tensor_sub(out=d_[c][:, :, 0:W - 1],
                                         in0=z[c][:, :, 2:W + 1],
                                         in1=z[c][:, :, 1:W])
                for c in range(NCH):
                    nc.vector.scalar_tensor_tensor(
                        out=vw[c][:, :, 0:W - 1], in0=d_[c][:, :, 0:W - 1],
                        scalar=SIGMA, in1=u2w[c][:, :, 1:W],
                        op0=OP.mult, op1=OP.add)
                if use_bf16:
                    vhsrc = psV
                else:
                    for c in range(NCH):
                        nc.vector.tensor_add(out=vh[c], in0=psV[c], in1=u2h[c])
                    vhsrc = vh
                for c in range(NCH):
                    nc.scalar.activation(out=hh[c], in_=vhsrc[c], func=AF.Square)
                for c in range(NCH):
                    nc.scalar.activation(out=ww[c], in_=vw[c], func=AF.Square)
                for c in range(NCH):
                    nc.gpsimd.tensor_add(out=n2[c], in0=hh[c], in1=ww[c])
                for c in range(NCH):
                    nc.vector.tensor_scalar(out=m_[c], in0=n2[c],
                                            scalar1=ths2[:, 0:1], scalar2=None,
                                            op0=OP.max)
                for c in range(NCH):
                    nc.scalar.activation(out=tl[c], in_=m_[c], func=AF.Ln)
                for c in range(NCH):
                    nc.scalar.activation(out=f_[c], in_=tl[c], func=AF.Exp,
                                         bias=lnt[:, 0:1], scale=-0.5)
                # ph/pw reuse the hh/ww buffers (dead after n2)
                for c in range(NCH):
                    nc.vector.scalar_tensor_tensor(
                        out=hh[c], in0=vhsrc[c], scalar=RHO, in1=f_[c],
                        op0=OP.mult, op1=OP.mult)
                for c in range(NCH):
                    nc.vector.scalar_tensor_tensor(
                        out=u2h[c], in0=u2h[c], scalar=1.0 - RHO,
                        in1=hh[c], op0=OP.mult, op1=OP.add)
                for c in range(NCH):
                    nc.vector.scalar_tensor_tensor(
                        out=ww[c], in0=vw[c], scalar=RHO, in1=f_[c],
                        op0=OP.mult, op1=OP.mult)
                for c in range(NCH):
                    nc.vector.scalar_tensor_tensor(
                        out=u2w[c][:, :, 1:W + 1], in0=u2w[c][:, :, 1:W + 1],
                        scalar=1.0 - RHO, in1=ww[c],
                        op0=OP.mult, op1=OP.add)

            # ---- iterations: hardware loop, 2 PDHG steps per body ----
            assert n_it % 2 == 0, n_it
            if unrolled:
                for k in range(n_it // 2):
                    half_iter(x2, x2o, f"u{2 * k}")
                    half_iter(x2o, x2, f"u{2 * k + 1}")
            elif n_it > 0:
                with tc.For_i(0, n_it // 2, 1):
                    half_iter(x2, x2o, 0)
                    half_iter(x2o, x2, 1)

            # ---- writeback ----
            for c in range(NCH):
                for b in range(BPC):
                    gb = BPC * c + b
                    nc.sync.dma_start(out=out_d[P * gb:P * (gb + 1), :],
                                      in_=x2[c][:, b, :])
    nc.compile()
    return nc


_CACHED = {}


def _make_runner(nc, n_cores):
    """Build a reusable jitted executor for nc (mirrors
    bass2jax.run_bass_via_pjrt, but the jax.jit object is created once so
    repeat calls skip retrace/recompile entirely)."""
    import jax
    from jax.experimental.shard_map import shard_map
    from jax.sharding import Mesh, PartitionSpec
    from concourse import bass2jax

    bass2jax.install_neuronx_cc_hook()
    assert nc.dbg_addr is None
    partition_name = (nc.partition_id_tensor.name
                      if nc.partition_id_tensor else None)
    in_names, out_names, out_avals, zero_specs = [], [], [], []
    for alloc in nc.m.functions[0].allocations:
        if not isinstance(alloc, mybir.MemoryLocationSet):
            continue
        name = alloc.memorylocations[0].name
        if alloc.kind == "ExternalInput":
            if name != partition_name:
                in_names.append(name)
        elif alloc.kind == "ExternalOutput":
            shape = tuple(alloc.tensor_shape)
            dtype = mybir.dt.np(alloc.dtype)
            out_names.append(name)
            out_avals.append(jax.core.ShapedArray(shape, dtype))
            zero_specs.append((shape, dtype))
    n_params = len(in_names)
    n_outs = len(out_avals)
    in_names = in_names + out_names
    if partition_name is not None:
        in_names.append(partition_name)
    donate = tuple(range(n_params, n_params + n_outs))

    def _body(*args):
        operands = list(args)
        if partition_name is not None:
            operands.append(bass2jax.partition_id_tensor())
        outs = bass2jax._bass_exec_p.bind(
            *operands,
            out_avals=tuple(out_avals),
            in_names=tuple(in_names),
            out_names=tuple(out_names),
            lowering_input_output_aliases=(),
            sim_require_finite=True,
            sim_require_nnan=True,
            nc=nc,
        )
        return tuple(outs)

    devices = jax.devices()[:n_cores]
    assert len(devices) == n_cores
    mesh = Mesh(np.asarray(devices), ("core",))
    in_specs = (PartitionSpec("core"),) * (n_params + n_outs)
    out_specs = (PartitionSpec("core"),) * n_outs
    sharded = jax.jit(
        shard_map(_body, mesh=mesh, in_specs=in_specs,
                  out_specs=out_specs, check_rep=False),
        donate_argnums=donate, keep_unused=True)

    def run(in_maps):
        per_core = [[np.asarray(m[name]) for name in in_names[:n_params]]
                    for m in in_maps]
        concat_in = [np.concatenate([per_core[c][i] for c in range(n_cores)],
                                    axis=0) for i in range(n_params)]
        concat_zeros = [np.zeros((n_cores * s[0], *s[1:]), d)
                        for (s, d) in zero_specs]
        out_arrs = sharded(*concat_in, *concat_zeros)
        return [{name: np.asarray(out_arrs[i]).reshape(n_cores,
                                                       *out_avals[i].shape)[c]
                 for i, name in enumerate(out_names)}
                for c in range(n_cores)]
    return run


def kernel(y: np.ndarray, ths: np.ndarray, n_it=N_IT) -> np.ndarray:
    y = np.ascontiguousarray(np.asarray(y, dtype=np.float32))
    B = y.shape[0]
    assert y.shape[1:] == (512, 512), y.shape
    key = ("run", n_it, USE_BF16, B)
    if key not in _CACHED:
        import time as _t
        _tb = _t.time()
        nc = build(n_it)
        _CACHED[key] = _make_runner(nc, B)
        print(f"[kernel] build({n_it}) took {_t.time()-_tb:.1f}s", flush=True)
    run = _CACHED[key]
    if USE_BF16:
        import ml_dtypes
        cst = _consts(ml_dtypes.bfloat16)
    else:
        cst = _consts(np.float32)
    onesrow = np.ones((1, P), dtype=np.float32)
    thsv = np.asarray(ths, dtype=np.float32).reshape(1, 1)
    in_maps = [{"y": y[i], "ths": thsv, "consts": cst, "onesrow": onesrow}
               for i in range(B)]
    import time as _t
    _tr = _t.time()
    results = run(in_maps)
    print(f"[kernel] run took {_t.time()-_tr:.1f}s", flush=True)
    out = np.stack([results[i]["out"] for i in range(B)])
    return out.astype(np.float32)


if __name__ == "__main__":
    rng = np.random.default_rng(0)
    y = rng.standard_normal((8, 512, 512), dtype=np.float32)
    out = kernel(y, np.float32(0.1))
    print("ran:", out.shape, out.dtype, float(np.abs(out).max()))

